# revision 16
# baseline (speedup 1.0000x reference)
"""BrainConstructor (topk_masking) TRN2 Bass kernel, SPMD over 8 NeuronCores.

Phase 1 (device, sharded over src rows): all-pairs edge scoring.  Each core
scores a [256, 2048] logits tile: ACT computes gelu(hs_i + ht_j + b1) with
d-on-partitions (two src rows packed per 128-partition instruction, hs_i+b1
applied as the per-partition activation bias), and PE contracts with w2 using
64 column-shifted stationary matrices that accumulate the 64 pair-blocks of a
128-src-row super-block into one dense [128, 2048] PSUM tile.

Host: K-th-value threshold with a safety margin picks ~30k candidate pairs
from the device logits; candidate logits are re-computed bit-exactly (eager
jax-CPU ops mirroring the reference, 3D-shaped dot) and stable-sorted to
reproduce the reference's exact top-k order.

Phase 2 (device, data-parallel over the K selected edges): re-score the
selected pairs through the edge scorer and projector MLPs (f32), producing
sigmoid probs and prob-scaled edge features.
"""

import numpy as np
import ml_dtypes

import concourse.bacc as bacc
import concourse.tile as tile
import concourse.mybir as mybir
from concourse.bass_utils import run_bass_kernel_spmd

F32 = mybir.dt.float32
BF16 = mybir.dt.bfloat16
AF = mybir.ActivationFunctionType

N = 2048
D = 64
DE = 32
K = 20961
N_CORES = 8
ROWS = N // N_CORES          # 256 src rows per core
PAIRS = ROWS // 2            # 128 pair-blocks per core
SUPERS = PAIRS // 64         # supers of 64 pairs (=128 psum partitions)
NPC = 2624                   # phase-2 pairs per core (8*2624 = 20992 >= K)
MARGIN = 0.08                # candidate threshold slack vs device logit error

_compiled = {}


def _build_phase1():
    R = 8                      # pair-blocks per ACT instruction
    nc = bacc.Bacc("TRN2", target_bir_lowering=False, debug=False)
    htT2 = nc.dram_tensor("htT2", [128, N], F32, kind="ExternalInput").ap()
    hsb = nc.dram_tensor("hsb", [128, PAIRS], F32, kind="ExternalInput").ap()
    w2s = nc.dram_tensor("w2s", [128, 64 * 128], BF16, kind="ExternalInput").ap()
    logits = nc.dram_tensor("logits", [ROWS, N], F32, kind="ExternalOutput").ap()

    with tile.TileContext(nc) as tc:
        with (
            tc.tile_pool(name="static", bufs=1) as static,
            tc.tile_pool(name="apool", bufs=2) as apool,
            tc.tile_pool(name="gpool", bufs=2) as gpool,
            tc.tile_pool(name="psum", bufs=2, space="PSUM") as psum,
            tc.tile_pool(name="lpool", bufs=4) as lpool,
        ):
            htT2_sb = static.tile([128, N], F32)
            nc.sync.dma_start(htT2_sb[:], htT2[:])
            hsb_sb = static.tile([128, PAIRS], F32)
            nc.sync.dma_start(hsb_sb[:], hsb[:])
            w2s_sb = static.tile([128, 64 * 128], BF16)
            nc.sync.dma_start(w2s_sb[:], w2s[:])

            for s in range(SUPERS):
                P_s = psum.tile([128, N], F32)
                for g in range(64 // R):
                    A_r = apool.tile([128, R * N], BF16)
                    for bl in range(R):
                        p = 64 * s + R * g + bl
                        nc.vector.tensor_scalar_add(
                            A_r[:, N * bl : N * (bl + 1)],
                            htT2_sb[:],
                            hsb_sb[:, p : p + 1],
                        )
                    G_r = gpool.tile([128, R * N], BF16)
                    nc.scalar.activation(G_r[:], A_r[:], AF.Gelu)
                    for bl in range(R):
                        b = R * g + bl
                        for q in range(4):
                            nc.tensor.matmul(
                                P_s[:, 512 * q : 512 * (q + 1)],
                                lhsT=w2s_sb[:, 128 * b : 128 * (b + 1)],
                                rhs=G_r[:, N * bl + 512 * q : N * bl + 512 * (q + 1)],
                                start=(b == 0),
                                stop=(b == 63),
                            )
                for q in range(4):
                    L_q = lpool.tile([128, 512], F32)
                    nc.vector.tensor_copy(L_q[:], P_s[:, 512 * q : 512 * (q + 1)])
                    nc.sync.dma_start(
                        logits[128 * s : 128 * (s + 1), 512 * q : 512 * (q + 1)],
                        L_q[:],
                    )

    nc.compile()
    return nc


def _build_phase2():
    """Single ACT table set (gelu_and_others): sigmoid(x) computed as
    0.5*tanh(x/2)+0.5.  W1 and Wp merged into one [128, 96] stationary so
    each chunk is one matmul + one gelu for both MLP branches."""
    npc = NPC
    CH = 512
    DH = D + DE                # 96 rows: h1 on [0:64), F on [64:96)
    chunks = [(c0, min(CH, npc - c0)) for c0 in range(0, npc, CH)]
    nc = bacc.Bacc("TRN2", target_bir_lowering=False, debug=False)
    selT = nc.dram_tensor("selT", [128, npc], F32, kind="ExternalInput").ap()
    w1p = nc.dram_tensor("w1p", [128, DH], F32, kind="ExternalInput").ap()
    w2c = nc.dram_tensor("w2c", [D, 1], F32, kind="ExternalInput").ap()
    b1p = nc.dram_tensor("b1p", [DH, 1], F32, kind="ExternalInput").ap()
    b2h = nc.dram_tensor("b2h", [1, 1], F32, kind="ExternalInput").ap()
    onesc = nc.dram_tensor("onesc", [1, DH], F32, kind="ExternalInput").ap()
    probs = nc.dram_tensor("probs", [1, npc], F32, kind="ExternalOutput").ap()
    featT = nc.dram_tensor("featT", [DE, npc], F32, kind="ExternalOutput").ap()

    with tile.TileContext(nc) as tc:
        with (
            tc.tile_pool(name="static", bufs=1) as static,
            tc.tile_pool(name="sbwork", bufs=1) as sbwork,
            tc.tile_pool(name="pa", bufs=2, space="PSUM") as pa,
        ):
            w1p_sb = static.tile([128, DH], F32)
            nc.sync.dma_start(w1p_sb[:], w1p[:])
            w2_sb = static.tile([D, 1], F32)
            nc.sync.dma_start(w2_sb[:], w2c[:])
            b1p_sb = static.tile([DH, 1], F32)
            nc.sync.dma_start(b1p_sb[:], b1p[:])
            b2h_sb = static.tile([1, 1], F32)
            nc.sync.dma_start(b2h_sb[:], b2h[:])
            ones_sb = static.tile([1, DH], F32)
            nc.sync.dma_start(ones_sb[:], onesc[:])
            selT_sb = static.tile([128, npc], F32)

            hf_sb = sbwork.tile([DH, npc], F32)
            th_sb = sbwork.tile([1, npc], F32)
            probs_sb = sbwork.tile([1, npc], F32)
            e_sb = sbwork.tile([DH, npc], F32)   # only [D:DH) used (DVE lane align)

            for c0, cw in chunks:
                sl = slice(c0, c0 + cw)
                nc.sync.dma_start(selT_sb[:, sl], selT[:, sl])
                p0 = pa.tile([DH, CH], F32, tag="p0")
                nc.tensor.matmul(p0[:, :cw], lhsT=w1p_sb[:], rhs=selT_sb[:, sl])
                nc.scalar.activation(
                    hf_sb[:, sl], p0[:, :cw], AF.Gelu, bias=b1p_sb[:]
                )
                pL = pa.tile([1, CH], F32, tag="pL")
                nc.tensor.matmul(pL[:, :cw], lhsT=w2_sb[:], rhs=hf_sb[:D, sl])
                # sigmoid(x + b2) == 0.5*tanh(0.5*x + 0.5*b2) + 0.5
                nc.scalar.activation(
                    th_sb[:, sl], pL[:, :cw], AF.Tanh, bias=b2h_sb[:], scale=0.5
                )
                nc.vector.tensor_scalar(
                    probs_sb[:, sl],
                    th_sb[:, sl],
                    0.5,
                    0.5,
                    mybir.AluOpType.mult,
                    mybir.AluOpType.add,
                )
                nc.sync.dma_start(probs[:, sl], probs_sb[:, sl])
                pD = pa.tile([DH, CH], F32, tag="pD")
                nc.tensor.matmul(pD[:, :cw], lhsT=ones_sb[:], rhs=probs_sb[:, sl])
                nc.vector.tensor_mul(
                    e_sb[D:DH, sl], hf_sb[D:DH, sl], pD[D:DH, :cw]
                )
                nc.sync.dma_start(featT[:, sl], e_sb[D:DH, sl])

    nc.compile()
    return nc


def _get(name, builder):
    if name not in _compiled:
        _compiled[name] = builder()
    return _compiled[name]


def _run_phase1(x, W1, b1, W2):
    nc = _get("phase1", _build_phase1)
    hs = x @ W1[:D]
    ht = x @ W1[D:]
    htT = np.ascontiguousarray(ht.T)
    htT2 = np.concatenate([htT, htT], axis=0)           # [128, N]
    hsb_all = (hs + b1).T                               # [64, N]

    w2 = W2[:, 0]
    w2s = np.zeros((128, 64 * 128), np.float32)
    for b in range(64):
        w2s[:D, 128 * b + 2 * b] = w2
        w2s[D:, 128 * b + 2 * b + 1] = w2
    w2s = w2s.astype(ml_dtypes.bfloat16)

    cols = np.arange(PAIRS)
    in_maps = []
    for c in range(N_CORES):
        i0 = ROWS * c
        hsb = np.empty((128, PAIRS), np.float32)
        hsb[:D] = hsb_all[:, i0 + 2 * cols]
        hsb[D:] = hsb_all[:, i0 + 2 * cols + 1]
        in_maps.append({"htT2": htT2, "hsb": hsb, "w2s": w2s})
    res = run_bass_kernel_spmd(nc, in_maps, core_ids=list(range(N_CORES)))
    return np.concatenate(
        [res.results[c]["logits"] for c in range(N_CORES)], axis=0
    )


def _select_topk(logits_dev, x, W1, b1, W2, b2):
    """Threshold candidates from device logits, then re-rank them with a
    bit-exact (vs the reference's eager jax-CPU ops) recompute."""
    import jax
    import jax.numpy as jnp

    fd = logits_dev.copy()
    np.fill_diagonal(fd, -np.inf)
    flat = fd.ravel()
    kth_dev = np.partition(flat, flat.size - K)[flat.size - K]
    cand = np.flatnonzero(flat >= kth_dev - MARGIN)
    ci = (cand // N).astype(np.int32)
    cj = (cand % N).astype(np.int32)

    C = cand.size
    C2 = 512
    C1 = -(-C // C2)
    pad = C1 * C2 - C
    ci_p = np.concatenate([ci, np.full(pad, ci[0], np.int32)])
    cj_p = np.concatenate([cj, np.full(pad, cj[0], np.int32)])

    cpu = jax.devices("cpu")[0]
    with jax.default_device(cpu):
        xj = jnp.asarray(x)
        W1j = jnp.asarray(W1)
        b1j = jnp.asarray(b1)
        W2j = jnp.asarray(W2)
        b2j = jnp.asarray(b2)
        hs = xj @ W1j[:D]
        ht = xj @ W1j[D:]
        a = hs[ci_p] + ht[cj_p] + b1j
        h = jax.nn.gelu(a.reshape(C1, C2, D), approximate=False)
        lo = (h @ W2j)[..., 0] + b2j[0]
        lo = np.asarray(lo).ravel()[:C]

    order = np.lexsort((cand, -lo))
    sel = cand[order[:K]]
    return (sel // N).astype(np.int32), (sel % N).astype(np.int32)


def _run_phase2(x, W1, b1, W2, b2, Wp, bp, src, tgt):
    nc = _get("phase2", _build_phase2)
    total = N_CORES * NPC
    pad = total - K
    src_p = np.concatenate([src, np.full(pad, src[0], np.int32)])
    tgt_p = np.concatenate([tgt, np.full(pad, tgt[0], np.int32)])
    xT = np.ascontiguousarray(x.T)
    selT_all = np.empty((128, total), np.float32)
    selT_all[:D] = xT[:, src_p]
    selT_all[D:] = xT[:, tgt_p]

    shared = {
        "w1p": np.ascontiguousarray(np.concatenate([W1, Wp], axis=1)),
        "w2c": W2,
        "b1p": np.concatenate([b1, bp]).reshape(D + DE, 1),
        "b2h": (0.5 * b2).reshape(1, 1),
        "onesc": np.ones((1, D + DE), np.float32),
    }
    in_maps = []
    for c in range(N_CORES):
        m = dict(shared)
        m["selT"] = np.ascontiguousarray(selT_all[:, c * NPC : (c + 1) * NPC])
        in_maps.append(m)
    res = run_bass_kernel_spmd(nc, in_maps, core_ids=list(range(N_CORES)))
    probs = np.concatenate(
        [res.results[c]["probs"][0] for c in range(N_CORES)]
    )[:K]
    featT = np.concatenate(
        [res.results[c]["featT"] for c in range(N_CORES)], axis=1
    )[:, :K]
    return probs, featT


def kernel(node_features, W1, b1, W2, b2, Wp, bp):
    x = np.ascontiguousarray(np.asarray(node_features, np.float32))
    W1 = np.ascontiguousarray(np.asarray(W1, np.float32))
    b1 = np.ascontiguousarray(np.asarray(b1, np.float32))
    W2 = np.ascontiguousarray(np.asarray(W2, np.float32))
    b2 = np.ascontiguousarray(np.asarray(b2, np.float32))
    Wp = np.ascontiguousarray(np.asarray(Wp, np.float32))
    bp = np.ascontiguousarray(np.asarray(bp, np.float32))

    logits_dev = _run_phase1(x, W1, b1, W2)
    src, tgt = _select_topk(logits_dev, x, W1, b1, W2, b2)
    probs, featT = _run_phase2(x, W1, b1, W2, b2, Wp, bp, src, tgt)

    edge_index = np.stack([src, tgt]).astype(np.int32)
    edge_features = np.ascontiguousarray(featT.T)
    confidence_loss = np.float32(np.mean(np.float32(1.0) - probs))
    return edge_index, edge_features, confidence_loss


# revision 17
# speedup vs baseline: 1.0078x; 1.0078x over previous
"""BrainConstructor (topk_masking) TRN2 Bass kernel, SPMD over 8 NeuronCores.

Phase 1 (device, sharded over src rows): all-pairs edge scoring.  Each core
scores a [256, 2048] logits tile: ACT computes gelu(hs_i + ht_j + b1) with
d-on-partitions (two src rows packed per 128-partition instruction, hs_i+b1
applied as the per-partition activation bias), and PE contracts with w2 using
64 column-shifted stationary matrices that accumulate the 64 pair-blocks of a
128-src-row super-block into one dense [128, 2048] PSUM tile.

Host: K-th-value threshold with a safety margin picks ~30k candidate pairs
from the device logits; candidate logits are re-computed bit-exactly (eager
jax-CPU ops mirroring the reference, 3D-shaped dot) and stable-sorted to
reproduce the reference's exact top-k order.

Phase 2 (device, data-parallel over the K selected edges): re-score the
selected pairs through the edge scorer and projector MLPs (f32), producing
sigmoid probs and prob-scaled edge features.
"""

import numpy as np
import ml_dtypes

import concourse.bacc as bacc
import concourse.tile as tile
import concourse.mybir as mybir
from concourse.bass_utils import run_bass_kernel_spmd

F32 = mybir.dt.float32
BF16 = mybir.dt.bfloat16
AF = mybir.ActivationFunctionType

N = 2048
D = 64
DE = 32
K = 20961
N_CORES = 8
ROWS = N // N_CORES          # 256 src rows per core
PAIRS = ROWS // 2            # 128 pair-blocks per core
SUPERS = PAIRS // 64         # supers of 64 pairs (=128 psum partitions)
NPC = 2624                   # phase-2 pairs per core (8*2624 = 20992 >= K)
MARGIN = 0.08                # candidate threshold slack vs device logit error

_compiled = {}


def _build_phase1():
    R = 8                      # pair-blocks per ACT instruction
    nc = bacc.Bacc("TRN2", target_bir_lowering=False, debug=False)
    htT2 = nc.dram_tensor("htT2", [128, N], F32, kind="ExternalInput").ap()
    hsb = nc.dram_tensor("hsb", [128, PAIRS], F32, kind="ExternalInput").ap()
    w2s = nc.dram_tensor("w2s", [128, 64 * 128], BF16, kind="ExternalInput").ap()
    logits = nc.dram_tensor("logits", [ROWS, N], F32, kind="ExternalOutput").ap()

    with tile.TileContext(nc) as tc:
        with (
            tc.tile_pool(name="static", bufs=1) as static,
            tc.tile_pool(name="apool", bufs=2) as apool,
            tc.tile_pool(name="gpool", bufs=2) as gpool,
            tc.tile_pool(name="psum", bufs=2, space="PSUM") as psum,
            tc.tile_pool(name="lpool", bufs=4) as lpool,
        ):
            htT2_sb = static.tile([128, N], F32)
            nc.sync.dma_start(htT2_sb[:], htT2[:])
            hsb_sb = static.tile([128, PAIRS], F32)
            nc.sync.dma_start(hsb_sb[:], hsb[:])
            w2s_sb = static.tile([128, 64 * 128], BF16)
            nc.sync.dma_start(w2s_sb[:], w2s[:])

            ramp_in = [1, 1, 2, 4] + [8] * 7
            groups_by_super = {0: ramp_in, SUPERS - 1: ramp_in[::-1]}
            for s in range(SUPERS):
                P_s = psum.tile([128, N], F32)
                b = 0
                for Rg in groups_by_super.get(s, [R] * (64 // R)):
                    A_r = apool.tile([128, R * N], BF16, tag="A")
                    for bl in range(Rg):
                        p = 64 * s + b + bl
                        nc.vector.tensor_scalar_add(
                            A_r[:, N * bl : N * (bl + 1)],
                            htT2_sb[:],
                            hsb_sb[:, p : p + 1],
                        )
                    G_r = gpool.tile([128, R * N], BF16, tag="G")
                    nc.scalar.activation(
                        G_r[:, : Rg * N], A_r[:, : Rg * N], AF.Gelu
                    )
                    for bl in range(Rg):
                        bb = b + bl
                        for q in range(4):
                            nc.tensor.matmul(
                                P_s[:, 512 * q : 512 * (q + 1)],
                                lhsT=w2s_sb[:, 128 * bb : 128 * (bb + 1)],
                                rhs=G_r[:, N * bl + 512 * q : N * bl + 512 * (q + 1)],
                                start=(bb == 0),
                                stop=(bb == 63),
                            )
                    b += Rg
                assert b == 64
                for q in range(4):
                    L_q = lpool.tile([128, 512], F32)
                    nc.vector.tensor_copy(L_q[:], P_s[:, 512 * q : 512 * (q + 1)])
                    nc.sync.dma_start(
                        logits[128 * s : 128 * (s + 1), 512 * q : 512 * (q + 1)],
                        L_q[:],
                    )

    nc.compile()
    return nc


def _build_phase2():
    """Single ACT table set (gelu_and_others): sigmoid(x) computed as
    0.5*tanh(x/2)+0.5.  W1 and Wp merged into one [128, 96] stationary so
    each chunk is one matmul + one gelu for both MLP branches."""
    npc = NPC
    CH = 512
    DH = D + DE                # 96 rows: h1 on [0:64), F on [64:96)
    chunks = [(c0, min(CH, npc - c0)) for c0 in range(0, npc, CH)]
    nc = bacc.Bacc("TRN2", target_bir_lowering=False, debug=False)
    selT = nc.dram_tensor("selT", [128, npc], F32, kind="ExternalInput").ap()
    w1p = nc.dram_tensor("w1p", [128, DH], F32, kind="ExternalInput").ap()
    w2c = nc.dram_tensor("w2c", [D, 1], F32, kind="ExternalInput").ap()
    b1p = nc.dram_tensor("b1p", [DH, 1], F32, kind="ExternalInput").ap()
    b2h = nc.dram_tensor("b2h", [1, 1], F32, kind="ExternalInput").ap()
    onesc = nc.dram_tensor("onesc", [1, DH], F32, kind="ExternalInput").ap()
    probs = nc.dram_tensor("probs", [1, npc], F32, kind="ExternalOutput").ap()
    featT = nc.dram_tensor("featT", [DE, npc], F32, kind="ExternalOutput").ap()

    with tile.TileContext(nc) as tc:
        with (
            tc.tile_pool(name="static", bufs=1) as static,
            tc.tile_pool(name="sbwork", bufs=1) as sbwork,
            tc.tile_pool(name="pa", bufs=2, space="PSUM") as pa,
        ):
            w1p_sb = static.tile([128, DH], F32)
            nc.sync.dma_start(w1p_sb[:], w1p[:])
            w2_sb = static.tile([D, 1], F32)
            nc.sync.dma_start(w2_sb[:], w2c[:])
            b1p_sb = static.tile([DH, 1], F32)
            nc.sync.dma_start(b1p_sb[:], b1p[:])
            b2h_sb = static.tile([1, 1], F32)
            nc.sync.dma_start(b2h_sb[:], b2h[:])
            ones_sb = static.tile([1, DH], F32)
            nc.sync.dma_start(ones_sb[:], onesc[:])
            selT_sb = static.tile([128, npc], F32)

            hf_sb = sbwork.tile([DH, npc], F32)
            th_sb = sbwork.tile([1, npc], F32)
            probs_sb = sbwork.tile([1, npc], F32)
            e_sb = sbwork.tile([DH, npc], F32)   # only [D:DH) used (DVE lane align)

            for c0, cw in chunks:
                sl = slice(c0, c0 + cw)
                nc.sync.dma_start(selT_sb[:, sl], selT[:, sl])
                p0 = pa.tile([DH, CH], F32, tag="p0")
                nc.tensor.matmul(p0[:, :cw], lhsT=w1p_sb[:], rhs=selT_sb[:, sl])
                nc.scalar.activation(
                    hf_sb[:, sl], p0[:, :cw], AF.Gelu, bias=b1p_sb[:]
                )
                pL = pa.tile([1, CH], F32, tag="pL")
                nc.tensor.matmul(pL[:, :cw], lhsT=w2_sb[:], rhs=hf_sb[:D, sl])
                # sigmoid(x + b2) == 0.5*tanh(0.5*x + 0.5*b2) + 0.5
                nc.scalar.activation(
                    th_sb[:, sl], pL[:, :cw], AF.Tanh, bias=b2h_sb[:], scale=0.5
                )
                nc.vector.tensor_scalar(
                    probs_sb[:, sl],
                    th_sb[:, sl],
                    0.5,
                    0.5,
                    mybir.AluOpType.mult,
                    mybir.AluOpType.add,
                )
                nc.sync.dma_start(probs[:, sl], probs_sb[:, sl])
                pD = pa.tile([DH, CH], F32, tag="pD")
                nc.tensor.matmul(pD[:, :cw], lhsT=ones_sb[:], rhs=probs_sb[:, sl])
                nc.vector.tensor_mul(
                    e_sb[D:DH, sl], hf_sb[D:DH, sl], pD[D:DH, :cw]
                )
                nc.sync.dma_start(featT[:, sl], e_sb[D:DH, sl])

    nc.compile()
    return nc


def _get(name, builder):
    if name not in _compiled:
        _compiled[name] = builder()
    return _compiled[name]


def _run_phase1(x, W1, b1, W2):
    nc = _get("phase1", _build_phase1)
    hs = x @ W1[:D]
    ht = x @ W1[D:]
    htT = np.ascontiguousarray(ht.T)
    htT2 = np.concatenate([htT, htT], axis=0)           # [128, N]
    hsb_all = (hs + b1).T                               # [64, N]

    w2 = W2[:, 0]
    w2s = np.zeros((128, 64 * 128), np.float32)
    for b in range(64):
        w2s[:D, 128 * b + 2 * b] = w2
        w2s[D:, 128 * b + 2 * b + 1] = w2
    w2s = w2s.astype(ml_dtypes.bfloat16)

    cols = np.arange(PAIRS)
    in_maps = []
    for c in range(N_CORES):
        i0 = ROWS * c
        hsb = np.empty((128, PAIRS), np.float32)
        hsb[:D] = hsb_all[:, i0 + 2 * cols]
        hsb[D:] = hsb_all[:, i0 + 2 * cols + 1]
        in_maps.append({"htT2": htT2, "hsb": hsb, "w2s": w2s})
    res = run_bass_kernel_spmd(nc, in_maps, core_ids=list(range(N_CORES)))
    return np.concatenate(
        [res.results[c]["logits"] for c in range(N_CORES)], axis=0
    )


def _select_topk(logits_dev, x, W1, b1, W2, b2):
    """Threshold candidates from device logits, then re-rank them with a
    bit-exact (vs the reference's eager jax-CPU ops) recompute."""
    import jax
    import jax.numpy as jnp

    fd = logits_dev.copy()
    np.fill_diagonal(fd, -np.inf)
    flat = fd.ravel()
    kth_dev = np.partition(flat, flat.size - K)[flat.size - K]
    cand = np.flatnonzero(flat >= kth_dev - MARGIN)
    ci = (cand // N).astype(np.int32)
    cj = (cand % N).astype(np.int32)

    C = cand.size
    C2 = 512
    C1 = -(-C // C2)
    pad = C1 * C2 - C
    ci_p = np.concatenate([ci, np.full(pad, ci[0], np.int32)])
    cj_p = np.concatenate([cj, np.full(pad, cj[0], np.int32)])

    cpu = jax.devices("cpu")[0]
    with jax.default_device(cpu):
        xj = jnp.asarray(x)
        W1j = jnp.asarray(W1)
        b1j = jnp.asarray(b1)
        W2j = jnp.asarray(W2)
        b2j = jnp.asarray(b2)
        hs = xj @ W1j[:D]
        ht = xj @ W1j[D:]
        a = hs[ci_p] + ht[cj_p] + b1j
        h = jax.nn.gelu(a.reshape(C1, C2, D), approximate=False)
        lo = (h @ W2j)[..., 0] + b2j[0]
        lo = np.asarray(lo).ravel()[:C]

    order = np.lexsort((cand, -lo))
    sel = cand[order[:K]]
    return (sel // N).astype(np.int32), (sel % N).astype(np.int32)


def _run_phase2(x, W1, b1, W2, b2, Wp, bp, src, tgt):
    nc = _get("phase2", _build_phase2)
    total = N_CORES * NPC
    pad = total - K
    src_p = np.concatenate([src, np.full(pad, src[0], np.int32)])
    tgt_p = np.concatenate([tgt, np.full(pad, tgt[0], np.int32)])
    xT = np.ascontiguousarray(x.T)
    selT_all = np.empty((128, total), np.float32)
    selT_all[:D] = xT[:, src_p]
    selT_all[D:] = xT[:, tgt_p]

    shared = {
        "w1p": np.ascontiguousarray(np.concatenate([W1, Wp], axis=1)),
        "w2c": W2,
        "b1p": np.concatenate([b1, bp]).reshape(D + DE, 1),
        "b2h": (0.5 * b2).reshape(1, 1),
        "onesc": np.ones((1, D + DE), np.float32),
    }
    in_maps = []
    for c in range(N_CORES):
        m = dict(shared)
        m["selT"] = np.ascontiguousarray(selT_all[:, c * NPC : (c + 1) * NPC])
        in_maps.append(m)
    res = run_bass_kernel_spmd(nc, in_maps, core_ids=list(range(N_CORES)))
    probs = np.concatenate(
        [res.results[c]["probs"][0] for c in range(N_CORES)]
    )[:K]
    featT = np.concatenate(
        [res.results[c]["featT"] for c in range(N_CORES)], axis=1
    )[:, :K]
    return probs, featT


def kernel(node_features, W1, b1, W2, b2, Wp, bp):
    x = np.ascontiguousarray(np.asarray(node_features, np.float32))
    W1 = np.ascontiguousarray(np.asarray(W1, np.float32))
    b1 = np.ascontiguousarray(np.asarray(b1, np.float32))
    W2 = np.ascontiguousarray(np.asarray(W2, np.float32))
    b2 = np.ascontiguousarray(np.asarray(b2, np.float32))
    Wp = np.ascontiguousarray(np.asarray(Wp, np.float32))
    bp = np.ascontiguousarray(np.asarray(bp, np.float32))

    logits_dev = _run_phase1(x, W1, b1, W2)
    src, tgt = _select_topk(logits_dev, x, W1, b1, W2, b2)
    probs, featT = _run_phase2(x, W1, b1, W2, b2, Wp, bp, src, tgt)

    edge_index = np.stack([src, tgt]).astype(np.int32)
    edge_features = np.ascontiguousarray(featT.T)
    confidence_loss = np.float32(np.mean(np.float32(1.0) - probs))
    return edge_index, edge_features, confidence_loss


# revision 23
# speedup vs baseline: 1.0086x; 1.0008x over previous
"""BrainConstructor (topk_masking) TRN2 Bass kernel, SPMD over 8 NeuronCores.

Phase 1 (device, sharded over src rows): all-pairs edge scoring.  Each core
scores a [256, 2048] logits tile: ACT computes gelu(hs_i + ht_j + b1) with
d-on-partitions (two src rows packed per 128-partition instruction, hs_i+b1
applied as the per-partition activation bias), and PE contracts with w2 using
64 column-shifted stationary matrices that accumulate the 64 pair-blocks of a
128-src-row super-block into one dense [128, 2048] PSUM tile.

Host: K-th-value threshold with a safety margin picks ~30k candidate pairs
from the device logits; candidate logits are re-computed bit-exactly (eager
jax-CPU ops mirroring the reference, 3D-shaped dot) and stable-sorted to
reproduce the reference's exact top-k order.

Phase 2 (device, data-parallel over the K selected edges): re-score the
selected pairs through the edge scorer and projector MLPs (f32), producing
sigmoid probs and prob-scaled edge features.
"""

import numpy as np
import ml_dtypes

import concourse.bacc as bacc
import concourse.tile as tile
import concourse.mybir as mybir
from concourse.bass_utils import run_bass_kernel_spmd

F32 = mybir.dt.float32
BF16 = mybir.dt.bfloat16
AF = mybir.ActivationFunctionType

N = 2048
D = 64
DE = 32
K = 20961
N_CORES = 8
ROWS = N // N_CORES          # 256 src rows per core
PAIRS = ROWS // 2            # 128 pair-blocks per core
SUPERS = PAIRS // 64         # supers of 64 pairs (=128 psum partitions)
NPC = 2624                   # phase-2 pairs per core (8*2624 = 20992 >= K)
MARGIN = 0.08                # candidate threshold slack vs device logit error

_compiled = {}


def _build_phase1():
    R = 8                      # pair-blocks per ACT instruction
    nc = bacc.Bacc("TRN2", target_bir_lowering=False, debug=False)
    htT2 = nc.dram_tensor("htT2", [128, N], F32, kind="ExternalInput").ap()
    hsb = nc.dram_tensor("hsb", [128, PAIRS], F32, kind="ExternalInput").ap()
    w2s = nc.dram_tensor("w2s", [128, 64 * 128], BF16, kind="ExternalInput").ap()
    logits = nc.dram_tensor("logits", [ROWS, N], F32, kind="ExternalOutput").ap()

    with tile.TileContext(nc) as tc:
        with (
            tc.tile_pool(name="static", bufs=1) as static,
            tc.tile_pool(name="apool", bufs=2) as apool,
            tc.tile_pool(name="gpool", bufs=2) as gpool,
            tc.tile_pool(name="psum", bufs=2, space="PSUM") as psum,
            tc.tile_pool(name="lpool", bufs=2) as lpool,
        ):
            htT2_sb = static.tile([128, N], F32)
            nc.sync.dma_start(htT2_sb[:], htT2[:])
            hsb_sb = static.tile([128, PAIRS], F32)
            nc.sync.dma_start(hsb_sb[:], hsb[:])
            w2s_sb = static.tile([128, 64 * 128], BF16)
            nc.sync.dma_start(w2s_sb[:], w2s[:])

            ramp_in = [1, 1, 2, 2, 2] + [8] * 7
            groups_by_super = {0: ramp_in, SUPERS - 1: ramp_in[::-1]}
            for s in range(SUPERS):
                P_s = psum.tile([128, N], F32)
                b = 0
                for Rg in groups_by_super.get(s, [R] * (64 // R)):
                    if Rg <= 2:
                        A_r = apool.tile([128, 2 * N], BF16, tag="As")
                        G_r = gpool.tile([128, 2 * N], BF16, tag="Gs")
                    else:
                        A_r = apool.tile([128, R * N], BF16, tag="A")
                        G_r = gpool.tile([128, R * N], BF16, tag="G")
                    for bl in range(Rg):
                        p = 64 * s + b + bl
                        nc.vector.tensor_scalar_add(
                            A_r[:, N * bl : N * (bl + 1)],
                            htT2_sb[:],
                            hsb_sb[:, p : p + 1],
                        )
                    nc.scalar.activation(
                        G_r[:, : Rg * N], A_r[:, : Rg * N], AF.Gelu
                    )
                    for bl in range(Rg):
                        bb = b + bl
                        for q in range(4):
                            nc.tensor.matmul(
                                P_s[:, 512 * q : 512 * (q + 1)],
                                lhsT=w2s_sb[:, 128 * bb : 128 * (bb + 1)],
                                rhs=G_r[:, N * bl + 512 * q : N * bl + 512 * (q + 1)],
                                start=(bb == 0),
                                stop=(bb == 63),
                            )
                    b += Rg
                assert b == 64
                for q in range(4):
                    L_q = lpool.tile([128, 512], F32)
                    if s == SUPERS - 1 and q % 2 == 1:
                        # ACT is drained by now; split the evacuation across
                        # both engines so the tail halves
                        nc.scalar.copy(L_q[:], P_s[:, 512 * q : 512 * (q + 1)])
                    else:
                        nc.vector.tensor_copy(
                            L_q[:], P_s[:, 512 * q : 512 * (q + 1)]
                        )
                    nc.sync.dma_start(
                        logits[128 * s : 128 * (s + 1), 512 * q : 512 * (q + 1)],
                        L_q[:],
                    )

    nc.compile()
    return nc


def _build_phase2():
    """Single ACT table set (gelu_and_others): sigmoid(x) computed as
    0.5*tanh(x/2)+0.5.  W1 and Wp merged into one [128, 96] stationary so
    each chunk is one matmul + one gelu for both MLP branches."""
    npc = NPC
    CH = 512
    DH = D + DE                # 96 rows: h1 on [0:64), F on [64:96)
    chunks = [(c0, min(CH, npc - c0)) for c0 in range(0, npc, CH)]
    nc = bacc.Bacc("TRN2", target_bir_lowering=False, debug=False)
    selT = nc.dram_tensor("selT", [128, npc], F32, kind="ExternalInput").ap()
    # packed: cols [0:DH)=W1|Wp, col DH=w2 (rows 0:64), col DH+1=b1|bp,
    # col DH+2 row 0 = 0.5*b2
    wpack = nc.dram_tensor("wpack", [128, DH + 3], F32, kind="ExternalInput").ap()
    probs = nc.dram_tensor("probs", [1, npc], F32, kind="ExternalOutput").ap()
    featT = nc.dram_tensor("featT", [DE, npc], F32, kind="ExternalOutput").ap()

    with tile.TileContext(nc) as tc:
        with (
            tc.tile_pool(name="static", bufs=1) as static,
            tc.tile_pool(name="sbwork", bufs=1) as sbwork,
            tc.tile_pool(name="pa", bufs=2, space="PSUM") as pa,
        ):
            wp_sb = static.tile([128, DH + 3], F32)
            nc.sync.dma_start(wp_sb[:], wpack[:])
            w1p_sb = wp_sb[:, 0:DH]
            w2_sb = wp_sb[0:D, DH : DH + 1]
            b1p_sb = wp_sb[0:DH, DH + 1 : DH + 2]
            b2h_sb = wp_sb[0:1, DH + 2 : DH + 3]
            ones_sb = static.tile([1, DH], F32)
            nc.vector.memset(ones_sb[:], 1.0)
            selT_sb = static.tile([128, npc], F32)

            hf_sb = sbwork.tile([DH, npc], F32)
            th_sb = sbwork.tile([1, npc], F32)
            probs_sb = sbwork.tile([1, npc], F32)
            e_sb = sbwork.tile([DH, npc], F32)   # only [D:DH) used (DVE lane align)

            for c0, cw in chunks:
                sl = slice(c0, c0 + cw)
                nc.sync.dma_start(selT_sb[:, sl], selT[:, sl])
                p0 = pa.tile([DH, CH], F32, tag="p0")
                nc.tensor.matmul(p0[:, :cw], lhsT=w1p_sb[:], rhs=selT_sb[:, sl])
                nc.scalar.activation(
                    hf_sb[:, sl], p0[:, :cw], AF.Gelu, bias=b1p_sb[:]
                )
                pL = pa.tile([1, CH], F32, tag="pL")
                nc.tensor.matmul(pL[:, :cw], lhsT=w2_sb[:], rhs=hf_sb[:D, sl])
                # sigmoid(x + b2) == 0.5*tanh(0.5*x + 0.5*b2) + 0.5
                nc.scalar.activation(
                    th_sb[:, sl], pL[:, :cw], AF.Tanh, bias=b2h_sb[:], scale=0.5
                )
                nc.vector.tensor_scalar(
                    probs_sb[:, sl],
                    th_sb[:, sl],
                    0.5,
                    0.5,
                    mybir.AluOpType.mult,
                    mybir.AluOpType.add,
                )
                nc.sync.dma_start(probs[:, sl], probs_sb[:, sl])
                pD = pa.tile([DH, CH], F32, tag="pD")
                nc.tensor.matmul(pD[:, :cw], lhsT=ones_sb[:], rhs=probs_sb[:, sl])
                nc.vector.tensor_mul(
                    e_sb[D:DH, sl], hf_sb[D:DH, sl], pD[D:DH, :cw]
                )
                nc.sync.dma_start(featT[:, sl], e_sb[D:DH, sl])

    nc.compile()
    return nc


def _get(name, builder):
    if name not in _compiled:
        _compiled[name] = builder()
    return _compiled[name]


def _run_phase1(x, W1, b1, W2):
    nc = _get("phase1", _build_phase1)
    hs = x @ W1[:D]
    ht = x @ W1[D:]
    htT = np.ascontiguousarray(ht.T)
    htT2 = np.concatenate([htT, htT], axis=0)           # [128, N]
    hsb_all = (hs + b1).T                               # [64, N]

    w2 = W2[:, 0]
    w2s = np.zeros((128, 64 * 128), np.float32)
    for b in range(64):
        w2s[:D, 128 * b + 2 * b] = w2
        w2s[D:, 128 * b + 2 * b + 1] = w2
    w2s = w2s.astype(ml_dtypes.bfloat16)

    cols = np.arange(PAIRS)
    in_maps = []
    for c in range(N_CORES):
        i0 = ROWS * c
        hsb = np.empty((128, PAIRS), np.float32)
        hsb[:D] = hsb_all[:, i0 + 2 * cols]
        hsb[D:] = hsb_all[:, i0 + 2 * cols + 1]
        in_maps.append({"htT2": htT2, "hsb": hsb, "w2s": w2s})
    res = run_bass_kernel_spmd(nc, in_maps, core_ids=list(range(N_CORES)))
    return np.concatenate(
        [res.results[c]["logits"] for c in range(N_CORES)], axis=0
    )


def _select_topk(logits_dev, x, W1, b1, W2, b2):
    """Threshold candidates from device logits, then re-rank them with a
    bit-exact (vs the reference's eager jax-CPU ops) recompute."""
    import jax
    import jax.numpy as jnp

    fd = logits_dev.copy()
    np.fill_diagonal(fd, -np.inf)
    flat = fd.ravel()
    kth_dev = np.partition(flat, flat.size - K)[flat.size - K]
    cand = np.flatnonzero(flat >= kth_dev - MARGIN)
    ci = (cand // N).astype(np.int32)
    cj = (cand % N).astype(np.int32)

    C = cand.size
    C2 = 512
    C1 = -(-C // C2)
    pad = C1 * C2 - C
    ci_p = np.concatenate([ci, np.full(pad, ci[0], np.int32)])
    cj_p = np.concatenate([cj, np.full(pad, cj[0], np.int32)])

    cpu = jax.devices("cpu")[0]
    with jax.default_device(cpu):
        xj = jnp.asarray(x)
        W1j = jnp.asarray(W1)
        b1j = jnp.asarray(b1)
        W2j = jnp.asarray(W2)
        b2j = jnp.asarray(b2)
        hs = xj @ W1j[:D]
        ht = xj @ W1j[D:]
        a = hs[ci_p] + ht[cj_p] + b1j
        h = jax.nn.gelu(a.reshape(C1, C2, D), approximate=False)
        lo = (h @ W2j)[..., 0] + b2j[0]
        lo = np.asarray(lo).ravel()[:C]

    order = np.lexsort((cand, -lo))
    sel = cand[order[:K]]
    return (sel // N).astype(np.int32), (sel % N).astype(np.int32)


def _run_phase2(x, W1, b1, W2, b2, Wp, bp, src, tgt):
    nc = _get("phase2", _build_phase2)
    total = N_CORES * NPC
    pad = total - K
    src_p = np.concatenate([src, np.full(pad, src[0], np.int32)])
    tgt_p = np.concatenate([tgt, np.full(pad, tgt[0], np.int32)])
    xT = np.ascontiguousarray(x.T)
    selT_all = np.empty((128, total), np.float32)
    selT_all[:D] = xT[:, src_p]
    selT_all[D:] = xT[:, tgt_p]

    DH = D + DE
    wpack = np.zeros((128, DH + 3), np.float32)
    wpack[:, 0:DH] = np.concatenate([W1, Wp], axis=1)
    wpack[0:D, DH] = W2[:, 0]
    wpack[0:DH, DH + 1] = np.concatenate([b1, bp])
    wpack[0, DH + 2] = 0.5 * float(b2[0])
    shared = {"wpack": wpack}
    in_maps = []
    for c in range(N_CORES):
        m = dict(shared)
        m["selT"] = np.ascontiguousarray(selT_all[:, c * NPC : (c + 1) * NPC])
        in_maps.append(m)
    res = run_bass_kernel_spmd(nc, in_maps, core_ids=list(range(N_CORES)))
    probs = np.concatenate(
        [res.results[c]["probs"][0] for c in range(N_CORES)]
    )[:K]
    featT = np.concatenate(
        [res.results[c]["featT"] for c in range(N_CORES)], axis=1
    )[:, :K]
    return probs, featT


def kernel(node_features, W1, b1, W2, b2, Wp, bp):
    x = np.ascontiguousarray(np.asarray(node_features, np.float32))
    W1 = np.ascontiguousarray(np.asarray(W1, np.float32))
    b1 = np.ascontiguousarray(np.asarray(b1, np.float32))
    W2 = np.ascontiguousarray(np.asarray(W2, np.float32))
    b2 = np.ascontiguousarray(np.asarray(b2, np.float32))
    Wp = np.ascontiguousarray(np.asarray(Wp, np.float32))
    bp = np.ascontiguousarray(np.asarray(bp, np.float32))

    logits_dev = _run_phase1(x, W1, b1, W2)
    src, tgt = _select_topk(logits_dev, x, W1, b1, W2, b2)
    probs, featT = _run_phase2(x, W1, b1, W2, b2, Wp, bp, src, tgt)

    edge_index = np.stack([src, tgt]).astype(np.int32)
    edge_features = np.ascontiguousarray(featT.T)
    confidence_loss = np.float32(np.mean(np.float32(1.0) - probs))
    return edge_index, edge_features, confidence_loss
